# revision 43
# baseline (speedup 1.0000x reference)
"""Trainium2 Bass kernel for nn_Attention_19739669692939 (sparse_attention).

Reference computation (shapes: L=1024, B=64, C=1024, D=512, E=512):
    Wa_e = W_attn[:, :C]        # [E, C]
    Wa_s = W_attn[:, C:]        # [E, D]
    pre  = enc_output @ Wa_e.T + s @ Wa_s.T     # [L, B, E] (s broadcast over L)
    engry = tanh(pre)
    att[b, l] = engry[l, b, :] @ W_v[0, :]
    out = softmax(att, axis=-1)                 # [B, 1024]

Distribution: pure data-parallel over batch. Core i handles batches
[8i, 8i+8); no collectives.

Kernel structure (v2 — vs the earlier kernel which ran ~194us):
- pre is computed as [l(128 part), e(512 free)] tiles: stationary = a
  128x128 chunk of enc^T, moving = Wa^T [c, e] (512-wide streams). The PE
  does ONLY these matmuls (110us = the bf16 stream roofline) plus a few
  one-time setup transposes.
- enc^T tiles come from the XBAR DMA-transpose (SBUF->SBUF, bf16) for
  most (lc, b) tiles, and from PE transposes for the rest — tuned so the
  DMA total (32MB f32 HBM reads + XBAR traffic) and the PE total balance.
- The W_v reduction is ONE fused DVE tensor_tensor_reduce per tile
  (engry * wv_broadcast, summed along free axis into att columns). In the
  old [e, l] layout this was 55us of extra PE matmul streams.
- bias (the s @ Wa_s.T term) is added by DVE as a [l-broadcast, e] tile
  per batch (precomputed once), since ACT bias is per-partition only.
- softmax epilogue: att lives as [128 l', (b, lc, ls)]; exp on ACT, the
  partition-sums via two tiny PE matmuls (ones / block-mask), reciprocal
  on DVE, broadcast back via a mask matmul, scale on ACT (per-partition),
  one PE transpose to [b-major, l] and a single 32KB store.
"""

import numpy as np

import concourse.bass as bass
import concourse.mybir as mybir
from concourse import bacc
from concourse.bass_utils import run_bass_kernel_spmd
from concourse.masks import make_identity
from concourse.tile import TileContext

F32 = mybir.dt.float32
BF16 = mybir.dt.bfloat16
AF = mybir.ActivationFunctionType
ALU = mybir.AluOpType

L = 1024          # enc length
B = 64            # global batch
BL = 8            # batch per core
C = 1024          # enc feature dim (2*enc_hid)
D = 512           # dec feature dim
E = 512           # engry dim
NCORES = 8

NCB = C // 128    # 8 c-blocks
NDB = D // 128    # 4 d-blocks
NEB = E // 128    # 4 e-blocks (r)
LCH = 512         # l-chunk per (lc, b) tile
NLC = L // LCH    # 2
KSUB = LCH // 128  # 4 l-subblocks
NWB = (C + D) // 128  # 12 blocks over W_attn's column axis

NT = NLC * BL     # 16 (lc, b) tiles
# Tiles whose enc^T comes from the XBAR DMA transpose; the rest use PE
# transposes. Measured: the DMA fabric moves ~415 GB/s of touched bytes
# (enc loads alone = 126us incl. W), an XBAR tile adds ~5.5us of fabric
# and a PE tile ~2.4us of PE (transposes pipeline at ~60-90ns cadence).
XBAR_TILES = frozenset({3, 5, 7, 9})
N_WARM = 16       # PE warmup transposes (p-state ramp during wnat load)
# Nearly all loads issue upfront: the gpsimd queue is in-order, so any
# dependent compute queued between load-issues would stall the SWDGE
# ring (descriptor processing runs on the gpsimd cores) and starve the
# whole pipeline. For the same reason gpsimd only gets per-tile compute
# for LATE tiles, whose work runs after the transfers have drained.
PF = 9            # load prefetch depth
GP_TILES = frozenset({10, 11, 12, 13, 14, 15})  # wv-mul on gpsimd (late)


def build_nc():
    nc = bacc.Bacc("TRN2", target_bir_lowering=False, debug=False)

    enc = nc.dram_tensor("enc_output", [L, BL, C], F32, kind="ExternalInput").ap()
    s = nc.dram_tensor("s", [1, BL, D], F32, kind="ExternalInput").ap()
    w_attn = nc.dram_tensor("W_attn", [E, C + D], F32, kind="ExternalInput").ap()
    w_v = nc.dram_tensor("W_v", [1, E], F32, kind="ExternalInput").ap()
    out = nc.dram_tensor("out", [BL, L], F32, kind="ExternalOutput").ap()

    with TileContext(nc) as tc:
        with (
            tc.tile_pool(name="consts", bufs=1) as consts,
            tc.tile_pool(name="nat", bufs=9) as nat_pool,
            tc.tile_pool(name="encT", bufs=5) as encT_pool,
            tc.tile_pool(name="tmp", bufs=2) as tmp_pool,
            tc.tile_pool(name="engry", bufs=2) as engry_pool,
            tc.tile_pool(name="scr", bufs=2) as scr_pool,
            tc.tile_pool(name="tp", bufs=3, space="PSUM") as tp_pool,
            tc.tile_pool(name="pre", bufs=2, space="PSUM") as pre_pool,
            tc.tile_pool(name="sp", bufs=1, space="PSUM") as sp_pool,
        ):
            # ---------------- constants ----------------
            ident = consts.tile([128, 128], F32, tag="ident")
            make_identity(nc, ident)
            identB = consts.tile([128, 128], BF16, tag="identB")
            nc.vector.tensor_copy(identB[:], ident[:])
            ones1_bf = consts.tile([1, 128], BF16, tag="ones1_bf")
            nc.vector.memset(ones1_bf[:], 1.0)
            onescol_f = consts.tile([128, 1], F32, tag="onescol_f")
            nc.vector.memset(onescol_f[:], 1.0)
            onesrow8 = consts.tile([1, 8], F32, tag="onesrow8")
            nc.vector.memset(onesrow8[:], 1.0)

            # ---------------- W load (first DMA) ----------------
            # wnat [e'(128), (r 4, w 12, cc 128)] bf16, cast during SWDGE load
            wnat = consts.tile([128, NEB * (C + D)], BF16, tag="wnat")
            for r in range(NEB):
                nc.gpsimd.dma_start(
                    out=wnat[:, r * (C + D):(r + 1) * (C + D)].rearrange(
                        "p (w c) -> p w c", w=NWB),
                    in_=w_attn[r * 128:(r + 1) * 128, :].rearrange(
                        "p (w c) -> p w c", w=NWB),
                )
            # s [8, 512] bf16
            s_sb = consts.tile([BL, D], BF16, tag="s_sb")
            nc.gpsimd.dma_start(out=s_sb[:], in_=s[0])

            # wv on the HWDGE queue (tiny)
            wv_sb = consts.tile([1, E], F32, tag="wv_sb")
            nc.sync.dma_start(out=wv_sb[:], in_=w_v[:])

            # ---------------- PE warmup (covers wnat latency) ----------------
            warm_ps = tp_pool.tile([128, 512], BF16, tag="tp")
            for _ in range(N_WARM):
                nc.tensor.transpose(warm_ps[:, :128], identB[:], identB[:])

            # ---------------- waT via PE transposes ----------------
            # waT [cc 128, (w 12, r 4, e' 128)]: waT[cc, w, r, e'] =
            # W[r*128+e', w*128+cc]
            waT = consts.tile([128, NWB, NEB, 128], BF16, tag="waT")
            for r in range(NEB):
                for wp in range(NWB // 4):
                    tpw = tp_pool.tile([128, 512], BF16, tag="tp",
                                       name=f"tpw_{r}_{wp}")
                    for j in range(4):
                        w = wp * 4 + j
                        nc.tensor.transpose(
                            tpw[:, j * 128:(j + 1) * 128],
                            wnat[:, r * (C + D) + w * 128:
                                 r * (C + D) + (w + 1) * 128],
                            identB[:],
                        )
                    for j in range(4):
                        w = wp * 4 + j
                        dst = waT[:, w, r, :]
                        src = tpw[:, j * 128:(j + 1) * 128]
                        if j % 2 == 0:
                            nc.vector.tensor_copy(dst, src)
                        else:
                            nc.scalar.copy(dst, src)

            # ---------------- s^T and bias ----------------
            # sT [d' 128, (dd 4, b 8)] bf16
            sT = consts.tile([128, NDB * BL], BF16, tag="sT")
            for dd in range(NDB):
                tps = tp_pool.tile([128, 512], BF16, tag="tp")
                nc.tensor.transpose(
                    tps[:, :BL],
                    s_sb[:, dd * 128:(dd + 1) * 128],
                    identB[:BL, :BL],
                )
                nc.vector.tensor_copy(sT[:, dd * BL:(dd + 1) * BL], tps[:, :BL])

            # bias_sb [e' 128, (r 4, b 8)] bf16 = Wa_s @ s.T per e-block
            bias_sb = consts.tile([128, NEB * BL], BF16, tag="bias_sb")
            for r in range(NEB):
                bps = sp_pool.tile([128, 512], F32, tag="sp")
                for dd in range(NDB):
                    nc.tensor.matmul(
                        bps[:, :BL],
                        lhsT=waT[:, NCB + dd, r, :],
                        rhs=sT[:, dd * BL:(dd + 1) * BL],
                        start=(dd == 0),
                        stop=(dd == NDB - 1),
                    )
                nc.vector.tensor_copy(bias_sb[:, r * BL:(r + 1) * BL], bps[:, :BL])

            # biasT_sb [b 8, (r, e') 512] bf16
            biasT_sb = consts.tile([BL, E], BF16, tag="biasT_sb")
            for r in range(NEB):
                tpb = tp_pool.tile([128, 512], BF16, tag="tp")
                nc.tensor.transpose(
                    tpb[:BL, :128],
                    bias_sb[:, r * BL:(r + 1) * BL],
                    identB[:],
                )
                nc.scalar.copy(biasT_sb[:, r * 128:(r + 1) * 128], tpb[:BL, :128])

            # bias_rep2 [128, (b 8, e 1024)] f32: per b, extract row b of
            # biasT (identity-column matmul), broadcast to 128 partitions
            # (rank-1 ones matmul), store twice (the consume ops run at
            # [128, 1024] = two l-subblocks per instruction)
            bias_rep2 = consts.tile([128, BL, 2 * E], F32, tag="bias_rep2")
            rowb_sb = consts.tile([1, BL * E], BF16, tag="rowb_sb")
            for b in range(BL):
                rbp = sp_pool.tile([128, 512], F32, tag="sp")
                nc.tensor.matmul(
                    rbp[:1, :],
                    lhsT=identB[:BL, b:b + 1],
                    rhs=biasT_sb[:],
                    start=True, stop=True,
                )
                nc.vector.tensor_copy(rowb_sb[:, b * E:(b + 1) * E], rbp[:1, :])
            for b in range(BL):
                brp = sp_pool.tile([128, 512], F32, tag="sp")
                nc.tensor.matmul(
                    brp[:],
                    lhsT=ones1_bf[:],
                    rhs=rowb_sb[:, b * E:(b + 1) * E],
                    start=True, stop=True,
                )
                if b % 2 == 0:
                    nc.vector.tensor_copy(bias_rep2[:, b, 0:E], brp[:])
                    nc.vector.tensor_copy(bias_rep2[:, b, E:2 * E], brp[:])
                else:
                    nc.scalar.copy(bias_rep2[:, b, 0:E], brp[:])
                    nc.scalar.copy(bias_rep2[:, b, E:2 * E], brp[:])

            # wv_rep [128, 512] bf16: wv broadcast to all partitions
            wv_bf = consts.tile([1, E], BF16, tag="wv_bf")
            nc.vector.tensor_copy(wv_bf[:], wv_sb[:])
            wvp = sp_pool.tile([128, 512], F32, tag="sp")
            nc.tensor.matmul(wvp[:], lhsT=ones1_bf[:], rhs=wv_bf[:],
                             start=True, stop=True)
            wv_rep2 = consts.tile([128, 2 * E], BF16, tag="wv_rep2")
            nc.vector.tensor_copy(wv_rep2[:, 0:E], wvp[:])
            nc.vector.tensor_copy(wv_rep2[:, E:2 * E], wvp[:])

            # att columns: attC [128 l', (b 8, lc 2, ls 4)] f32
            attC = consts.tile([128, BL * NLC * KSUB], F32, tag="attC")

            # ---------------- main loop ----------------
            loads = {}

            def issue_load(t):
                lc, b = divmod(t, BL)
                nat_t = nat_pool.tile([128, KSUB * C], BF16, tag="nat",
                                      name=f"nat{t}")
                # 4KB-contiguous source descriptors (c unsplit)
                nc.gpsimd.dma_start(
                    out=nat_t.rearrange("p (k c) -> p k c", k=KSUB),
                    in_=enc[lc * LCH:(lc + 1) * LCH, b, :].rearrange(
                        "(k p) c -> p k c", p=128),
                )
                loads[t] = nat_t

            for t in range(PF):
                issue_load(t)

            # per-tile state for software pipelining: encT(t+1) is produced
            # interleaved with tile t's matmuls, at k-burst granularity, so
            # the PE always has a dense instruction stream and the copies
            # never queue behind the consume chain.
            encTs = {}

            def alloc_encT(t):
                encT = encT_pool.tile([128, KSUB, NCB, 128], BF16, tag="encT",
                                      name=f"encT{t}")
                encTs[t] = encT
                return encT

            def emit_xbar(t):
                nat_t = loads.pop(t)
                encT = alloc_encT(t)
                nc.sync.dma_start(
                    out=encT.rearrange("p k w l -> p (k w) l"),
                    in_=nat_t[:],
                    transpose=True,
                )

            def emit_tburst(t, k, nat_v):
                """8 PE transposes + 1 copy for k-subblock of tile t."""
                encT = encTs[t]
                tpt = tp_pool.tile([128, 1024], BF16, tag="tp",
                                   name=f"tpt{t}_{k}")
                for cb in range(NCB):
                    nc.tensor.transpose(
                        tpt[:, cb * 128:(cb + 1) * 128],
                        nat_v[:, k, cb, :],
                        identB[:],
                    )
                nc.scalar.copy(encT[:, k, :, :], tpt[:])

            # prime: produce encT(0) fully, start encT(1)
            if 0 in XBAR_TILES:
                emit_xbar(0)
            else:
                nat_v0 = loads.pop(0).rearrange("p (k w c) -> p k w c",
                                                k=KSUB, w=NCB)
                alloc_encT(0)
                for k in range(KSUB):
                    emit_tburst(0, k, nat_v0)

            xbar_done = set()

            for t in range(NT):
                lc, b = divmod(t, BL)
                encT = encTs.pop(t)
                if t + PF < NT:
                    issue_load(t + PF)
                # XBAR tiles are produced two tiles ahead (they take a full
                # load+transpose round trip on the DMA fabric; one tile of
                # lookahead stalls the PE)
                for tt in (t + 1, t + 2):
                    if tt < NT and tt in XBAR_TILES and tt not in xbar_done:
                        emit_xbar(tt)
                        xbar_done.add(tt)
                tn = t + 1
                nat_vn = None
                if tn < NT and tn not in XBAR_TILES:
                    nat_vn = loads.pop(tn).rearrange(
                        "p (k w c) -> p k w c", k=KSUB, w=NCB)
                    alloc_encT(tn)

                for pair in range(KSUB // 2):
                    pre = pre_pool.tile([128, 2 * E], F32, tag="pre",
                                        name=f"pre{t}_{pair}")
                    for half in range(2):
                        ls = pair * 2 + half
                        if nat_vn is not None:
                            emit_tburst(tn, ls, nat_vn)
                        for cb in range(NCB):
                            nc.tensor.matmul(
                                pre[:, half * E:(half + 1) * E],
                                lhsT=encT[:, ls, cb, :],
                                rhs=waT[:, cb, :, :],
                                start=(cb == 0),
                                stop=(cb == NCB - 1),
                            )
                    # consume at [128, 1024] granularity (2 l-subblocks)
                    tmp = tmp_pool.tile([128, 2 * E], F32, tag="tmp",
                                        name=f"tmp{t}_{pair}")
                    eng = engry_pool.tile([128, 2 * E], BF16, tag="engry",
                                          name=f"eng{t}_{pair}")
                    scr = scr_pool.tile([128, 2 * E], BF16, tag="scr",
                                        name=f"scr{t}_{pair}")
                    col = b * (NLC * KSUB) + lc * KSUB + pair * 2
                    nc.vector.tensor_add(tmp[:], pre[:], bias_rep2[:, b, :])
                    nc.scalar.activation(eng[:], tmp[:], AF.Tanh)
                    if t in GP_TILES:
                        nc.gpsimd.tensor_mul(scr[:], eng[:], wv_rep2[:])
                    else:
                        nc.vector.tensor_mul(scr[:], eng[:], wv_rep2[:])
                    nc.vector.reduce_sum(
                        attC[:, col:col + 2],
                        scr.rearrange("p (two e) -> p two e", two=2),
                        axis=mybir.AxisListType.X)

            # ---------------- softmax epilogue ----------------
            # logits bounded by ||W_v||_1 ~ 18 -> skip max subtraction
            attE = consts.tile([128, BL * NLC * KSUB], F32, tag="attE")
            nc.scalar.activation(attE[:], attC[:], AF.Exp)

            # per-(b,lc,ls) sums over l' via ones matmul, then transpose to a
            # row, group-reduce to per-b totals, reciprocal, expand back to
            # [64, 1] per-partition scales via a transpose
            sums_ps = sp_pool.tile([128, 512], F32, tag="sp")
            nc.tensor.matmul(sums_ps[:64, :1], lhsT=attE[:], rhs=onescol_f[:],
                             start=True, stop=True)
            sums_sb = consts.tile([64, 1], F32, tag="sums_sb")
            nc.vector.tensor_copy(sums_sb[:], sums_ps[:64, :1])

            sumsT_ps = sp_pool.tile([128, 512], F32, tag="sp")
            nc.tensor.transpose(sumsT_ps[:1, :64], sums_sb[:], ident[:64, :64])
            sumsT_sb = consts.tile([1, 64], F32, tag="sumsT_sb")
            nc.vector.tensor_copy(sumsT_sb[:], sumsT_ps[:1, :64])

            totT = consts.tile([1, BL], F32, tag="totT")
            nc.vector.reduce_sum(
                totT[:],
                sumsT_sb.rearrange("p (b k) -> p b k", b=BL),
                axis=mybir.AxisListType.X,
            )
            recipT = consts.tile([1, BL], F32, tag="recipT")
            nc.vector.reciprocal(recipT[:], totT[:])
            recipRow = consts.tile([1, 64], F32, tag="recipRow")
            for b in range(BL):
                nc.vector.tensor_scalar_mul(
                    recipRow[:, b * BL:(b + 1) * BL], onesrow8[:],
                    recipT[:, b:b + 1],
                )
            recipB_ps = sp_pool.tile([128, 512], F32, tag="sp")
            nc.tensor.transpose(recipB_ps[:64, :1], recipRow[:], ident[:1, :1])
            recipB_sb = consts.tile([64, 1], F32, tag="recipB_sb")
            nc.vector.tensor_copy(recipB_sb[:], recipB_ps[:64, :1])

            attT_ps = sp_pool.tile([128, 512], F32, tag="sp")
            nc.tensor.transpose(attT_ps[:64, :128], attE[:], ident[:])
            attF = consts.tile([64, 128], F32, tag="attF")
            nc.scalar.activation(attF[:], attT_ps[:64, :128], AF.Copy,
                                 scale=recipB_sb[:])
            nc.sync.dma_start(
                out=out.rearrange("b (lc ls f) -> (b lc ls) f", lc=NLC, ls=KSUB),
                in_=attF[:],
            )

    nc.compile()
    return nc


_NC_CACHE = None


def _get_nc():
    global _NC_CACHE
    if _NC_CACHE is None:
        _NC_CACHE = build_nc()
    return _NC_CACHE


def make_in_maps(enc_output, s, W_attn, W_v):
    enc_output = np.asarray(enc_output, dtype=np.float32)
    s = np.asarray(s, dtype=np.float32)
    W_attn = np.ascontiguousarray(np.asarray(W_attn, dtype=np.float32))
    W_v = np.ascontiguousarray(np.asarray(W_v, dtype=np.float32))
    in_maps = []
    for i in range(NCORES):
        in_maps.append({
            "enc_output": np.ascontiguousarray(enc_output[:, i * BL:(i + 1) * BL, :]),
            "s": np.ascontiguousarray(s[:, i * BL:(i + 1) * BL, :]),
            "W_attn": W_attn,
            "W_v": W_v,
        })
    return in_maps


def kernel(enc_output, s, W_attn, W_v):
    nc = _get_nc()
    in_maps = make_in_maps(enc_output, s, W_attn, W_v)
    res = run_bass_kernel_spmd(nc, in_maps, core_ids=list(range(NCORES)))
    return np.concatenate([res.results[i]["out"] for i in range(NCORES)], axis=0)
